# revision 13
# baseline (speedup 1.0000x reference)
"""Trainium2 Bass kernel for: cummax(W) ++ cummax(H) -> Linear(2C, C).

Reference semantics (shapes hardcoded):
    grid [16, 128, 128, 256] f32
    xc = cummax(grid, axis=2)   # along W
    yc = cummax(grid, axis=1)   # along H
    out = concat([xc, yc], -1) @ W[512, 256] + b[256]    # [16, 128, 128, 256]

Strategy: data-parallel over batch (2 batches / core on 8 cores).
Host pre-transposes grid to channels-first [b, c_half, c, h, w] so that
on-chip tiles are [c(128 partitions), h*w(free)].  Per 16-row chunk:
  - W-cummax: one segmented `tensor_tensor_scan` per c-half
    (op0=add with a -1e38 mask at w=0 resets the running max per row).
  - H-cummax: row-recurrence y[h] = max(y[h-1], g[h]) via tensor_tensor.
  - Matmul: out[c_half, pix] += W_k[feat, c].T @ X_k[feat, pix], fp32,
    4 K-chunks (xc0, xc1, yc0, yc1) accumulated in PSUM, N=512 per MM.
  - Bias is added during the ScalarE PSUM->SBUF copy (activation Copy).
Output DRAM layout is [b, c_half, c, h*w]; host transposes back.
"""

import numpy as np

import concourse.bass as bass
import concourse.tile as tile
from concourse import bacc, mybir
from concourse.bass_utils import run_bass_kernel_spmd

B, H, W_DIM, C = 16, 128, 128, 256
N_CORES = 8
NB = B // N_CORES          # batches per core
P = 128                    # partitions
NCH = C // P               # channel halves (2)
KT = (2 * C) // P          # K chunks for the matmul (4)
HC = 16                    # h rows per streamed chunk
NEG = -1.0e38

FP32 = mybir.dt.float32


def build(nb=NB, h=H, hc=HC):
    """Build the per-core Bass program (same program on all cores)."""
    Alu = mybir.AluOpType
    nchunk = h // hc
    pix = hc * W_DIM           # pixels per chunk
    hw = h * W_DIM

    nc = bacc.Bacc("TRN2", target_bir_lowering=False, debug=False)
    grid_t = nc.dram_tensor(
        "grid_t", [nb, NCH, P, h, W_DIM], FP32, kind="ExternalInput"
    ).ap()
    w_t = nc.dram_tensor("w_t", [NCH, KT, P, P], FP32, kind="ExternalInput").ap()
    b_t = nc.dram_tensor("b_t", [P, NCH], FP32, kind="ExternalInput").ap()
    out_t = nc.dram_tensor("out_t", [nb, NCH, P, hw], FP32, kind="ExternalOutput").ap()

    with tile.TileContext(nc) as tc:
        with (
            tc.tile_pool(name="consts", bufs=1) as consts,
            tc.tile_pool(name="gin", bufs=3) as gin,
            tc.tile_pool(name="xcp", bufs=2) as xcp,
            tc.tile_pool(name="ycp", bufs=3) as ycp,
            tc.tile_pool(name="outp", bufs=2) as outp,
            tc.tile_pool(name="psum", bufs=4, space="PSUM") as psump,
        ):
            # Weights as 8 stationary [feat, c] tiles, indexed ch_out*KT + k.
            w_sb = consts.tile([P, NCH * KT, P], FP32)
            nc.sync.dma_start(out=w_sb, in_=w_t.rearrange("ch k f c -> f (ch k) c"))
            b_sb = consts.tile([P, NCH], FP32)
            nc.sync.dma_start(out=b_sb, in_=b_t)
            # ACT touches b_sb once so later drains never carry the DMA wait
            # (the activation struct has a single sync-wait slot).
            b_scratch = consts.tile([P, NCH], FP32)
            nc.scalar.copy(out=b_scratch, in_=b_sb)
            # Segmented-scan mask: -1e38 at w=0 of every row, 0 elsewhere.
            mask = consts.tile([P, hc, W_DIM], FP32)
            nc.vector.memset(mask, 0.0)
            nc.vector.memset(mask[:, :, 0:1], NEG)
            mask_f = mask.rearrange("p h w -> p (h w)")



            y_prev = None
            for b in range(nb):
                for j in range(nchunk):
                    g = gin.tile([P, NCH, hc, W_DIM], FP32)
                    nc.sync.dma_start(
                        out=g,
                        in_=grid_t[b].rearrange("c p h w -> p c h w")[
                            :, :, j * hc : (j + 1) * hc, :
                        ],
                    )
                    g_f = g.rearrange("p c h w -> p c (h w)")

                    # --- W-cummax: segmented scan along the flat (h, w) dim.
                    x = xcp.tile([P, NCH, pix], FP32)
                    # The scan lowers to S2S2D2_STT, which has very few
                    # sync-wait slots; absorb the cross-engine waits (g DMA,
                    # x slot release) into a cheap DVE copy first.
                    nc.vector.tensor_copy(x[:, :, 0:1], g_f[:, :, 0:1])
                    for ch in range(NCH):
                        nc.vector.tensor_tensor_scan(
                            out=x[:, ch, :],
                            data0=mask_f,
                            data1=g_f[:, ch, :],
                            initial=NEG,
                            op0=Alu.add,
                            op1=Alu.max,
                        )

                    # --- H-cummax: row recurrence (both c-halves per op).
                    y = ycp.tile([P, NCH, hc, W_DIM], FP32)
                    for hh in range(hc):
                        if j == 0 and hh == 0:
                            nc.vector.tensor_copy(y[:, :, 0, :], g[:, :, 0, :])
                        else:
                            prev = (
                                y[:, :, hh - 1, :]
                                if hh > 0
                                else y_prev[:, :, hc - 1, :]
                            )
                            nc.vector.tensor_tensor(
                                y[:, :, hh, :], g[:, :, hh, :], prev, Alu.max
                            )
                    y_prev = y
                    y_f = y.rearrange("p c h w -> p c (h w)")

                    # --- Matmul + bias. K order: xc0, xc1, yc0, yc1.
                    # o has a 1-element pad: an ACT "touch" writes it so the
                    # slot-release hazard (out-DMA) lands on the ACT clock
                    # without overlapping the drains' writes (overlapping
                    # same-engine WAW costs an extra sync-wait slot).
                    o_raw = outp.tile([P, NCH * pix + 1], FP32)
                    nc.scalar.copy(
                        out=o_raw[:, NCH * pix : NCH * pix + 1], in_=b_sb[:, 0:1]
                    )
                    o = o_raw[:, 0 : NCH * pix].rearrange(
                        "p (c x) -> p c x", c=NCH
                    )
                    for s in range(pix // 1024):
                        for ch_out in range(NCH):
                            pt = psump.tile([P, 1024], FP32, tag="pt")
                            # N=1 dummy matmul: re-points the PSUM
                            # slot-release hazard (ACT) onto the PE clock, so
                            # real matmuls and the drain each carry a single
                            # sync wait (the LDW/AC structs allow only one).
                            # The first one also absorbs the w_sb DMA wait.
                            nc.tensor.matmul(
                                pt[:, 0:1],
                                w_sb[:, 0, :],
                                w_sb[:, 0, 0:1],
                                start=True,
                                stop=True,
                            )
                            for pg in range(2):
                                lo = s * 1024 + pg * 512
                                for k in range(KT):
                                    src = x if k < NCH else y_f
                                    rhs = src[:, k % NCH, lo : lo + 512]
                                    nc.tensor.matmul(
                                        pt[:, pg * 512 : (pg + 1) * 512],
                                        w_sb[:, ch_out * KT + k, :],
                                        rhs,
                                        start=(k == 0),
                                        stop=(k == KT - 1),
                                    )
                            nc.scalar.activation(
                                out=o[:, ch_out, s * 1024 : (s + 1) * 1024],
                                in_=pt,
                                func=mybir.ActivationFunctionType.Identity,
                                bias=b_sb[:, ch_out : ch_out + 1],
                                scale=1.0,
                            )

                    nc.sync.dma_start(
                        out=out_t[b].rearrange("c p x -> p c x")[
                            :, :, j * pix : (j + 1) * pix
                        ],
                        in_=o,
                    )
    nc.compile()
    return nc


_built = {}


def _get_nc():
    if "nc" not in _built:
        _built["nc"] = build()
    return _built["nc"]


def make_in_maps(grid, Wm, bv):
    """Host-side shard + layout transform. Returns per-core input maps."""
    grid = np.ascontiguousarray(grid, dtype=np.float32)
    Wm = np.asarray(Wm, dtype=np.float32)
    bv = np.asarray(bv, dtype=np.float32)
    w_t = np.ascontiguousarray(Wm.reshape(KT, P, NCH, P).transpose(2, 0, 1, 3))
    b_t = np.ascontiguousarray(bv.reshape(NCH, P).T)
    in_maps = []
    for i in range(N_CORES):
        gc = grid[i * NB : (i + 1) * NB]  # [NB, H, W, C]
        gt = np.ascontiguousarray(gc.transpose(0, 3, 1, 2)).reshape(
            NB, NCH, P, H, W_DIM
        )
        in_maps.append({"grid_t": gt, "w_t": w_t, "b_t": b_t})
    return in_maps


def assemble_output(results):
    """Per-core [NB, NCH, P, H*W] -> full [B, H, W, C]."""
    outs = []
    for i in range(N_CORES):
        ot = results[i]["out_t"]
        oc = (
            ot.reshape(NB, NCH, P, H, W_DIM)
            .transpose(0, 3, 4, 1, 2)
            .reshape(NB, H, W_DIM, C)
        )
        outs.append(oc)
    return np.ascontiguousarray(np.concatenate(outs, axis=0))


def run(inputs, **kwargs):
    """Run on hardware; returns (output, BassKernelResults)."""
    nc = _get_nc()
    in_maps = make_in_maps(inputs["grid"], inputs["W"], inputs["b"])
    res = run_bass_kernel_spmd(nc, in_maps, core_ids=list(range(N_CORES)), **kwargs)
    return assemble_output(res.results), res


def kernel(**inputs) -> np.ndarray:
    out, _ = run(inputs)
    return out


# revision 17
# speedup vs baseline: 1.8428x; 1.8428x over previous
"""Trainium2 Bass kernel for: cummax(W) ++ cummax(H) -> Linear(2C, C).

Reference semantics (shapes hardcoded):
    grid [16, 128, 128, 256] f32
    xc = cummax(grid, axis=2)   # along W
    yc = cummax(grid, axis=1)   # along H
    out = concat([xc, yc], -1) @ W[512, 256] + b[256]    # [16, 128, 128, 256]

Strategy: data-parallel over batch (2 batches / core on 8 cores).
Host pre-transposes grid to channels-first [b, c_half, c, h, w] so that
on-chip tiles are [c(128 partitions), h*w(free)].  Per 16-row chunk:
  - W-cummax: one segmented `tensor_tensor_scan` per c-half
    (op0=add with a -1e38 mask at w=0 resets the running max per row).
  - H-cummax: row-recurrence y[h] = max(y[h-1], g[h]) via tensor_tensor.
  - Matmul: out[c_half, pix] += W_k[feat, c].T @ X_k[feat, pix], fp32,
    4 K-chunks (xc0, xc1, yc0, yc1) accumulated in PSUM, N=512 per MM.
  - Bias is added during the ScalarE PSUM->SBUF copy (activation Copy).
Output DRAM layout is [b, c_half, c, h*w]; host transposes back.
"""

import numpy as np

import concourse.bass as bass
import concourse.tile as tile
from concourse import bacc, mybir
from concourse.bass_utils import run_bass_kernel_spmd

B, H, W_DIM, C = 16, 128, 128, 256
N_CORES = 8
NB = B // N_CORES          # batches per core
P = 128                    # partitions
NCH = C // P               # channel halves (2)
KT = (2 * C) // P          # K chunks for the matmul (4)
HC = 16                    # h rows per streamed chunk
NEG = -1.0e38

FP32 = mybir.dt.float32
FP16 = mybir.dt.float16


def build(nb=NB, h=H, hc=HC):
    """Build the per-core Bass program (same program on all cores)."""
    Alu = mybir.AluOpType
    nchunk = h // hc
    pix = hc * W_DIM           # pixels per chunk
    hw = h * W_DIM

    nc = bacc.Bacc("TRN2", target_bir_lowering=False, debug=False)
    grid_t = nc.dram_tensor(
        "grid_t", [nb, NCH, P, h, W_DIM], FP16, kind="ExternalInput"
    ).ap()
    w_t = nc.dram_tensor("w_t", [NCH, KT, P, P], FP16, kind="ExternalInput").ap()
    b_t = nc.dram_tensor("b_t", [P, NCH], FP32, kind="ExternalInput").ap()
    out_t = nc.dram_tensor("out_t", [nb, NCH, P, hw], FP32, kind="ExternalOutput").ap()

    with tile.TileContext(nc) as tc:
        with (
            tc.tile_pool(name="consts", bufs=1) as consts,
            tc.tile_pool(name="gin", bufs=3) as gin,
            tc.tile_pool(name="xcp", bufs=2) as xcp,
            tc.tile_pool(name="ycp", bufs=3) as ycp,
            tc.tile_pool(name="outp", bufs=2) as outp,
            tc.tile_pool(name="psum", bufs=4, space="PSUM") as psump,
        ):
            # Weights as 8 stationary [feat, c] tiles, indexed ch_out*KT + k.
            w_sb = consts.tile([P, NCH * KT, P], FP16)
            nc.sync.dma_start(out=w_sb, in_=w_t.rearrange("ch k f c -> f (ch k) c"))
            b_sb = consts.tile([P, NCH], FP32)
            nc.sync.dma_start(out=b_sb, in_=b_t)
            # ACT touches b_sb once so later drains never carry the DMA wait
            # (the activation struct has a single sync-wait slot).
            b_scratch = consts.tile([P, NCH], FP32)
            nc.scalar.copy(out=b_scratch, in_=b_sb)
            # Segmented-scan mask: -1e38 at w=0 of every row, 0 elsewhere.
            mask = consts.tile([P, hc, W_DIM], FP32)
            nc.vector.memset(mask, 0.0)
            nc.vector.memset(mask[:, :, 0:1], NEG)
            mask_f = mask.rearrange("p h w -> p (h w)")



            y_prev = None
            for b in range(nb):
                for j in range(nchunk):
                    g = gin.tile([P, NCH, hc, W_DIM], FP16)
                    nc.sync.dma_start(
                        out=g,
                        in_=grid_t[b].rearrange("c p h w -> p c h w")[
                            :, :, j * hc : (j + 1) * hc, :
                        ],
                    )
                    g_f = g.rearrange("p c h w -> p c (h w)")

                    # --- W-cummax: segmented scan along the flat (h, w) dim.
                    x = xcp.tile([P, NCH, pix], FP16)
                    # The scan lowers to S2S2D2_STT, which has very few
                    # sync-wait slots; absorb the cross-engine waits (g DMA,
                    # x slot release) into a cheap DVE copy first.
                    nc.vector.tensor_copy(x[:, :, 0:1], g_f[:, :, 0:1])
                    for ch in range(NCH):
                        nc.vector.tensor_tensor_scan(
                            out=x[:, ch, :],
                            data0=mask_f,
                            data1=g_f[:, ch, :],
                            initial=NEG,
                            op0=Alu.add,
                            op1=Alu.max,
                        )

                    # --- H-cummax: row recurrence (both c-halves per op).
                    y = ycp.tile([P, NCH, hc, W_DIM], FP16)
                    for hh in range(hc):
                        if j == 0 and hh == 0:
                            nc.vector.tensor_copy(y[:, :, 0, :], g[:, :, 0, :])
                        else:
                            prev = (
                                y[:, :, hh - 1, :]
                                if hh > 0
                                else y_prev[:, :, hc - 1, :]
                            )
                            nc.vector.tensor_tensor(
                                y[:, :, hh, :], g[:, :, hh, :], prev, Alu.max
                            )
                    y_prev = y
                    y_f = y.rearrange("p c h w -> p c (h w)")

                    # --- Matmul + bias. K order: xc0, xc1, yc0, yc1.
                    # o has a 1-element pad: an ACT "touch" writes it so the
                    # slot-release hazard (out-DMA) lands on the ACT clock
                    # without overlapping the drains' writes (overlapping
                    # same-engine WAW costs an extra sync-wait slot).
                    o_raw = outp.tile([P, NCH * pix + 1], FP32)
                    nc.scalar.copy(
                        out=o_raw[:, NCH * pix : NCH * pix + 1], in_=b_sb[:, 0:1]
                    )
                    o = o_raw[:, 0 : NCH * pix].rearrange(
                        "p (c x) -> p c x", c=NCH
                    )
                    for s in range(pix // 1024):
                        for ch_out in range(NCH):
                            pt = psump.tile([P, 1024], FP32, tag="pt")
                            # N=1 dummy matmul: re-points the PSUM
                            # slot-release hazard (ACT) onto the PE clock, so
                            # real matmuls and the drain each carry a single
                            # sync wait (the LDW/AC structs allow only one).
                            # The first one also absorbs the w_sb DMA wait.
                            nc.tensor.matmul(
                                pt[:, 0:1],
                                w_sb[:, 0, :],
                                w_sb[:, 0, 0:1],
                                start=True,
                                stop=True,
                            )
                            for pg in range(2):
                                lo = s * 1024 + pg * 512
                                for k in range(KT):
                                    src = x if k < NCH else y_f
                                    rhs = src[:, k % NCH, lo : lo + 512]
                                    nc.tensor.matmul(
                                        pt[:, pg * 512 : (pg + 1) * 512],
                                        w_sb[:, ch_out * KT + k, :],
                                        rhs,
                                        start=(k == 0),
                                        stop=(k == KT - 1),
                                    )
                            nc.scalar.activation(
                                out=o[:, ch_out, s * 1024 : (s + 1) * 1024],
                                in_=pt,
                                func=mybir.ActivationFunctionType.Identity,
                                bias=b_sb[:, ch_out : ch_out + 1],
                                scale=1.0,
                            )

                    nc.sync.dma_start(
                        out=out_t[b].rearrange("c p x -> p c x")[
                            :, :, j * pix : (j + 1) * pix
                        ],
                        in_=o,
                    )
    nc.compile()
    return nc


_built = {}


def _get_nc():
    if "nc" not in _built:
        _built["nc"] = build()
    return _built["nc"]


def make_in_maps(grid, Wm, bv):
    """Host-side shard + layout transform. Returns per-core input maps."""
    grid = np.asarray(grid, dtype=np.float32).astype(np.float16)
    Wm = np.asarray(Wm, dtype=np.float32).astype(np.float16)
    bv = np.asarray(bv, dtype=np.float32)
    w_t = np.ascontiguousarray(Wm.reshape(KT, P, NCH, P).transpose(2, 0, 1, 3))
    b_t = np.ascontiguousarray(bv.reshape(NCH, P).T)
    in_maps = []
    for i in range(N_CORES):
        gc = grid[i * NB : (i + 1) * NB]  # [NB, H, W, C]
        gt = np.ascontiguousarray(gc.transpose(0, 3, 1, 2)).reshape(
            NB, NCH, P, H, W_DIM
        )
        in_maps.append({"grid_t": gt, "w_t": w_t, "b_t": b_t})
    return in_maps


def assemble_output(results):
    """Per-core [NB, NCH, P, H*W] -> full [B, H, W, C]."""
    outs = []
    for i in range(N_CORES):
        ot = results[i]["out_t"]
        oc = (
            ot.reshape(NB, NCH, P, H, W_DIM)
            .transpose(0, 3, 4, 1, 2)
            .reshape(NB, H, W_DIM, C)
        )
        outs.append(oc)
    return np.ascontiguousarray(np.concatenate(outs, axis=0))


def run(inputs, **kwargs):
    """Run on hardware; returns (output, BassKernelResults)."""
    nc = _get_nc()
    in_maps = make_in_maps(inputs["grid"], inputs["W"], inputs["b"])
    res = run_bass_kernel_spmd(nc, in_maps, core_ids=list(range(N_CORES)), **kwargs)
    return assemble_output(res.results), res


def kernel(**inputs) -> np.ndarray:
    out, _ = run(inputs)
    return out


# revision 20
# speedup vs baseline: 2.1536x; 1.1687x over previous
"""Trainium2 Bass kernel for: cummax(W) ++ cummax(H) -> Linear(2C, C).

Reference semantics (shapes hardcoded):
    grid [16, 128, 128, 256] f32
    xc = cummax(grid, axis=2)   # along W
    yc = cummax(grid, axis=1)   # along H
    out = concat([xc, yc], -1) @ W[512, 256] + b[256]    # [16, 128, 128, 256]

Strategy: data-parallel over batch (2 batches / core on 8 cores).
Host pre-transposes grid to channels-first fp16 [c_half, c, b, h, w] so
on-chip tiles are [c(128 partitions), (c_half, b, h, w) free].  Both
per-core batches are processed together in 8-row chunks:
  - W-cummax: one segmented `tensor_tensor_scan` per c-half over the
    flat (b, h, w) dim (op0=add with a -1e38 mask at w=0 resets the
    running max at each row start).
  - H-cummax: row-recurrence y[h] = max(y[h-1], g[h]) via fp16
    tensor_tensor at FD=512 (both c-halves x both batches per op).
  - Matmul: out[c_half, pix] += W_k[feat, c].T @ X_k[feat, pix], fp16
    operands, fp32 PSUM, 4 K-chunks (xc0, xc1, yc0, yc1), N=512 per MM.
  - Bias is added during the ScalarE PSUM->SBUF copy; output is stored
    fp16 (host upcasts) to halve output DMA.
Everything fp16 on chip: monotone rounding commutes with cummax, and
the matmul accumulates in fp32, so rel err ~4e-4 vs the fp32 reference.
"""

import numpy as np

import concourse.tile as tile
from concourse import bacc, mybir
from concourse.bass_utils import run_bass_kernel_spmd

B, H, W_DIM, C = 16, 128, 128, 256
N_CORES = 8
NB = B // N_CORES          # batches per core
P = 128                    # partitions
NCH = C // P               # channel halves (2)
KT = (2 * C) // P          # K chunks for the matmul (4)
HC = 8                     # h rows per streamed chunk (both batches)
NEG = -1.0e38

FP32 = mybir.dt.float32
FP16 = mybir.dt.float16


def build(nb=NB, h=H, hc=HC):
    """Build the per-core Bass program (same program on all cores)."""
    Alu = mybir.AluOpType
    nchunk = h // hc
    cpix = nb * hc * W_DIM     # flat (b, h, w) pixels per chunk
    hw = h * W_DIM

    nc = bacc.Bacc("TRN2", target_bir_lowering=False, debug=False)
    grid_t = nc.dram_tensor(
        "grid_t", [NCH, P, nb, h, W_DIM], FP16, kind="ExternalInput"
    ).ap()
    w_t = nc.dram_tensor("w_t", [NCH, KT, P, P], FP16, kind="ExternalInput").ap()
    b_t = nc.dram_tensor("b_t", [P, NCH], FP32, kind="ExternalInput").ap()
    out_t = nc.dram_tensor(
        "out_t", [NCH, P, nb, hw], FP16, kind="ExternalOutput"
    ).ap()

    with tile.TileContext(nc) as tc:
        with (
            tc.tile_pool(name="consts", bufs=1) as consts,
            tc.tile_pool(name="gin", bufs=3) as gin,
            tc.tile_pool(name="xcp", bufs=2) as xcp,
            tc.tile_pool(name="ycp", bufs=3) as ycp,
            tc.tile_pool(name="outp", bufs=3) as outp,
            tc.tile_pool(name="psum", bufs=4, space="PSUM") as psump,
        ):
            # Weights as 8 stationary [feat, c] tiles, indexed ch_out*KT + k.
            w_sb = consts.tile([P, NCH * KT, P], FP16)
            nc.sync.dma_start(out=w_sb, in_=w_t.rearrange("ch k f c -> f (ch k) c"))
            b_sb = consts.tile([P, NCH], FP32)
            nc.sync.dma_start(out=b_sb, in_=b_t)
            # ACT touches b_sb once so later drains never carry the DMA wait
            # (the activation struct has a single sync-wait slot).
            b_scratch = consts.tile([P, NCH], FP32)
            nc.scalar.copy(out=b_scratch, in_=b_sb)
            # Segmented-scan mask: -1e38 at w=0 of every row, 0 elsewhere.
            mask = consts.tile([P, nb * hc, W_DIM], FP32)
            nc.vector.memset(mask, 0.0)
            nc.vector.memset(mask[:, :, 0:1], NEG)
            mask_f = mask.rearrange("p r w -> p (r w)")

            y_prev = None
            for j in range(nchunk):
                g = gin.tile([P, NCH, nb, hc, W_DIM], FP16)
                for ch in range(NCH):
                    nc.sync.dma_start(
                        out=g[:, ch],
                        in_=grid_t[ch].rearrange("p b h w -> p b h w")[
                            :, :, j * hc : (j + 1) * hc, :
                        ],
                    )
                g_f = g.rearrange("p c b h w -> p c (b h w)")

                # --- W-cummax: segmented scan along the flat (b, h, w) dim.
                x = xcp.tile([P, NCH, cpix], FP16)
                # The scan lowers to S2S2D2_STT, which has very few
                # sync-wait slots; absorb the cross-engine waits (g DMA,
                # x slot release) into a cheap DVE copy first.
                nc.vector.tensor_copy(x[:, :, 0:1], g_f[:, :, 0:1])
                for ch in range(NCH):
                    nc.vector.tensor_tensor_scan(
                        out=x[:, ch, :],
                        data0=mask_f,
                        data1=g_f[:, ch, :],
                        initial=NEG,
                        op0=Alu.add,
                        op1=Alu.max,
                    )

                # --- H-cummax: row recurrence (c-halves x batches per op).
                y = ycp.tile([P, NCH, nb, hc, W_DIM], FP16)
                for hh in range(hc):
                    if j == 0 and hh == 0:
                        nc.vector.tensor_copy(y[:, :, :, 0, :], g[:, :, :, 0, :])
                    else:
                        prev = (
                            y[:, :, :, hh - 1, :]
                            if hh > 0
                            else y_prev[:, :, :, hc - 1, :]
                        )
                        nc.vector.tensor_tensor(
                            y[:, :, :, hh, :], g[:, :, :, hh, :], prev, Alu.max
                        )
                y_prev = y
                y_f = y.rearrange("p c b h w -> p c (b h w)")

                # --- Matmul + bias. K order: xc0, xc1, yc0, yc1.
                # o has a 1-element pad: an ACT "touch" writes it so the
                # slot-release hazard (out-DMA) lands on the ACT clock
                # without overlapping the drains' writes (overlapping
                # same-engine WAW costs an extra sync-wait slot).
                o_raw = outp.tile([P, NCH * cpix + 1], FP16)
                nc.scalar.copy(
                    out=o_raw[:, NCH * cpix : NCH * cpix + 1],
                    in_=w_sb[:, 0, 0:1],
                )
                o = o_raw[:, 0 : NCH * cpix].rearrange("p (c x) -> p c x", c=NCH)
                for s in range(cpix // 1024):
                    for ch_out in range(NCH):
                        pt = psump.tile([P, 1024], FP32, tag="pt")
                        # N=1 dummy matmul: re-points the PSUM slot-release
                        # hazard (ACT) onto the PE clock, so real matmuls
                        # and the drain each carry a single sync wait (the
                        # LDW/AC structs allow only one).  The first one
                        # also absorbs the w_sb DMA wait.
                        nc.tensor.matmul(
                            pt[:, 0:1],
                            w_sb[:, 0, :],
                            w_sb[:, 0, 0:1],
                            start=True,
                            stop=True,
                        )
                        for pg in range(2):
                            lo = s * 1024 + pg * 512
                            for k in range(KT):
                                src = x if k < NCH else y_f
                                rhs = src[:, k % NCH, lo : lo + 512]
                                nc.tensor.matmul(
                                    pt[:, pg * 512 : (pg + 1) * 512],
                                    w_sb[:, ch_out * KT + k, :],
                                    rhs,
                                    start=(k == 0),
                                    stop=(k == KT - 1),
                                )
                        nc.scalar.activation(
                            out=o[:, ch_out, s * 1024 : (s + 1) * 1024],
                            in_=pt,
                            func=mybir.ActivationFunctionType.Identity,
                            bias=b_sb[:, ch_out : ch_out + 1],
                            scale=1.0,
                        )

                o_4d = o.rearrange("p c (b x) -> p c b x", b=nb)
                for ch in range(NCH):
                    nc.sync.dma_start(
                        out=out_t[ch].rearrange("p b x -> p b x")[
                            :, :, j * hc * W_DIM : (j + 1) * hc * W_DIM
                        ],
                        in_=o_4d[:, ch],
                    )
    nc.compile()
    return nc


_built = {}


def _get_nc():
    if "nc" not in _built:
        _built["nc"] = build()
    return _built["nc"]


def make_in_maps(grid, Wm, bv):
    """Host-side shard + layout transform. Returns per-core input maps."""
    grid = np.asarray(grid, dtype=np.float32).astype(np.float16)
    Wm = np.asarray(Wm, dtype=np.float32).astype(np.float16)
    bv = np.asarray(bv, dtype=np.float32)
    w_t = np.ascontiguousarray(Wm.reshape(KT, P, NCH, P).transpose(2, 0, 1, 3))
    b_t = np.ascontiguousarray(bv.reshape(NCH, P).T)
    in_maps = []
    for i in range(N_CORES):
        gc = grid[i * NB : (i + 1) * NB]  # [NB, H, W, C]
        # -> [ch, c, b, h, w]
        gt = np.ascontiguousarray(gc.transpose(3, 0, 1, 2)).reshape(
            NCH, P, NB, H, W_DIM
        )
        in_maps.append({"grid_t": gt, "w_t": w_t, "b_t": b_t})
    return in_maps


def assemble_output(results):
    """Per-core [NCH, P, NB, H*W] fp16 -> full [B, H, W, C] fp32."""
    outs = []
    for i in range(N_CORES):
        ot = results[i]["out_t"]
        oc = (
            ot.reshape(NCH, P, NB, H, W_DIM)
            .transpose(2, 3, 4, 0, 1)
            .reshape(NB, H, W_DIM, C)
            .astype(np.float32)
        )
        outs.append(oc)
    return np.ascontiguousarray(np.concatenate(outs, axis=0))


def run(inputs, **kwargs):
    """Run on hardware; returns (output, BassKernelResults)."""
    nc = _get_nc()
    in_maps = make_in_maps(inputs["grid"], inputs["W"], inputs["b"])
    res = run_bass_kernel_spmd(nc, in_maps, core_ids=list(range(N_CORES)), **kwargs)
    return assemble_output(res.results), res


def kernel(**inputs) -> np.ndarray:
    out, _ = run(inputs)
    return out
